# revision 38
# baseline (speedup 1.0000x reference)
"""Trainium2 Bass kernel for nn_Attention_81037442941065.

Dual-attention module (spatial [b,h,n,n] + channel [b,h,d,d]) with
B=2, N=2048, DIM=1024, 16 heads of d=64.

Sharding: 8 cores = (2 batches) x (4 head-groups of 4 heads).
Each core computes its batch/head-group slice end-to-end and produces a
partial (over head groups) output projection; the host sums the 4 group
partials per batch (the "all-reduce after to_out") and adds b_out.

Key engine balance (per core the spatial softmax needs 16.8M exps; the
ScalarE ACT unit does 1 elem/cycle/lane => ~110us if done there alone,
which was the old bottleneck):
  - exp is SPLIT between ScalarE (exact, activation Exp) and VectorE
    (fast-exp bit trick: P = bitcast_bf16(int16(round(S' + B))) where
    S' = S * 128*log2(e)*scale is pre-scaled by folding the constant
    into the z1 projection output. +-3% multiplicative ripple, zero mean
    in log space; softmax normalization cancels the common mode.
  - 1/den uses the custom-DVE reciprocal_approx_fast (the exact
    nc.vector.reciprocal took 3.3us per [1,512] call).
  - final projection contracts head-PAIRS: cat/w_out stored as
    [128, .] pair-tiles so the contraction K=128 (full PE rows),
    halving the number of projection matmuls.
  - ~44 warmup matmuls on the (already loaded) w_out tiles keep the PE
    HAM activity monitor busy during the initial input DMA so the whole
    kernel runs at 2.4 GHz instead of spending its first ~37us at 1.2.
  - outputs are fp16 partials (halves the output DMA; host sums in f32).

Per-core layouts (everything "T" is [channels, tokens]):
  z1T, yhT   : [256, 2048] as 2 tiles [128, 2048] (head pair per tile);
               z1T is pre-scaled by FE_ALPHA (see above)
  xh_aug     : 16 tiles [128, 260] (natural layout per 128-token chunk;
               per head 65 cols = 64 channels + a ones column so the AV
               matmul also produces the softmax denominators)
  spatial    : S^T = yh @ z1'^T computed [keys, queries] as row-tiled
               concurrent head-pair matmuls; exp split ScalarE/VectorE;
               AV matmul lhsT=[xh|1] accumulates over key chunks ->
               rows 0..63 = unnormalized out1^T, row 64 = sum of exp.
  channel    : [64,64] per head, softmax via Exp+accum_out and
               reciprocal_approx_fast.
"""

import sys

for _p in ("/opt/trn_rl_repo", "/opt/pypackages"):
    if _p not in sys.path:
        sys.path.insert(0, _p)

import ml_dtypes
import numpy as np
from contextlib import ExitStack

import concourse.bacc as bacc
import concourse.mybir as mybir
import concourse.tile as tile
from concourse.tile import add_dep_helper
from concourse.bass_utils import run_bass_kernel_spmd

F32 = mybir.dt.float32
F16 = mybir.dt.float16
I16 = mybir.dt.int16
BF16 = mybir.dt.bfloat16
ATT = mybir.dt.bfloat16   # attention-internal matmul dtype
EXP = mybir.ActivationFunctionType.Exp

B, N, DIM = 2, 2048, 1024
HEADS, DH = 16, 64
G = 4              # head groups == cores per batch
HG = HEADS // G    # heads per group (4)
CIN = HG * DH      # inner channels per core (256)
NCORES = 8
KC = DIM // 128    # contraction chunks for projections (8)
NCH = N // 128     # 128-token chunks (16)
SCALE = DH ** -0.5            # 1/8
CM_SCALE = SCALE / (N / DH)   # 1/256

# fast-exp constants: S' = FE_ALPHA * S accumulates in PSUM; then
#   ScalarE path: exp(LN2_128 * S') == exp(SCALE * S)
#   VectorE path: bitcast_bf16(int16(S' + FE_B)) ~= exp(SCALE * S)
FE_ALPHA = 128.0 * 1.4426950408889634 * SCALE
LN2_128 = 0.6931471805599453 / 128.0
FE_B = 16256.0 - 7.33        # mean-zero-in-log mantissa tweak

# exp tile assignment: whole [128,1024] tiles (per-instruction overhead
# is ~270ns, so wider is cheaper) alternate by j parity: even j ->
# ScalarE exact exp, odd j -> VectorE fast-exp.
def _exp_on_dve(j):
    return j % 2 == 1 and j != 15

N_WARMUP = 60                # HAM warmup matmuls at kernel start
N_TAILFILL = 34              # keep-warm matmuls during the last tail chain


def _build_program():
    nc = bacc.Bacc(
        "TRN2", target_bir_lowering=False, debug=False, num_devices=NCORES
    )

    # ---- DRAM I/O ----
    xT_d = nc.dram_tensor("xT", [DIM, N], BF16, kind="ExternalInput").ap()
    yT_d = nc.dram_tensor("yT", [DIM, N], BF16, kind="ExternalInput").ap()
    zT_d = nc.dram_tensor("zT", [DIM, N], BF16, kind="ExternalInput").ap()
    wsa1_d = nc.dram_tensor("w_sa1", [DIM, CIN], BF16, kind="ExternalInput").ap()
    wsa2_d = nc.dram_tensor("w_sa2", [DIM, CIN], BF16, kind="ExternalInput").ap()
    wse1_d = nc.dram_tensor("w_se1", [DIM, CIN], BF16, kind="ExternalInput").ap()
    wse2_d = nc.dram_tensor("w_se2", [DIM, CIN], BF16, kind="ExternalInput").ap()
    wout_d = nc.dram_tensor("w_out", [CIN, DIM], ATT, kind="ExternalInput").ap()
    outT_d = nc.dram_tensor("outT", [DIM, N], F16, kind="ExternalOutput").ap()

    with tile.TileContext(nc) as tc, ExitStack() as ctx:
        ppool = ctx.enter_context(tc.tile_pool(name="persist", bufs=1))

        # Persistent projection outputs (live across both scopes).
        z1T = [ppool.tile([128, N], ATT, tag=f"z1T{m}", name=f"z1T{m}")
               for m in range(2)]
        yhT = [ppool.tile([128, N], ATT, tag=f"yhT{m}", name=f"yhT{m}")
               for m in range(2)]
        xh_aug = [ppool.tile([128, HG * (DH + 1)], ATT, tag=f"xa{i}",
                             name=f"xa{i}") for i in range(NCH)]
        secm_sb = [ppool.tile([128, DH], ATT, tag=f"cm{p}", name=f"cm{p}")
                   for p in range(2)]
        rs = [ppool.tile([128, 1], F32, tag=f"rs{q}", name=f"rs{q}")
              for q in range(2)]
        rs0 = ppool.tile([64, 1], F32, tag="rsz", name="rsz")
        rcm = [ppool.tile([128, 1], F32, tag=f"rcm{q}", name=f"rcm{q}")
               for q in range(2)]
        rcm0 = ppool.tile([64, 1], F32, tag="rcmz", name="rcmz")

        ptpool = ctx.enter_context(tc.tile_pool(name="pt", bufs=3))
        tpool = ctx.enter_context(tc.tile_pool(name="tails", bufs=3))
        opool = ctx.enter_context(tc.tile_pool(name="oout", bufs=3))
        spool = ctx.enter_context(tc.tile_pool(name="spat", bufs=1))
        # w_out as two head-pair tiles [128, DIM] matching catP below;
        # DMA'd first so they double as the HAM-warmup matmul operands.
        wqP = [spool.tile([128, DIM], ATT, tag=f"wq{p}", name=f"wq{p}")
               for p in range(2)]
        for p in range(2):
            nc.sync.dma_start(wqP[p][:], wout_d[p * 128:(p + 1) * 128, :])
        # cat^T staging as head-PAIR tiles [128, N]: head 2p+hh occupies
        # partitions 64*hh..64*hh+64.  The final projection contracts a
        # pair in ONE K=128 matmul.  DVE lanes are partition-locked, so
        # the odd head's blocks are computed at base 0 and DMA'd into
        # partitions 64:128 (DMA can shift partitions; DVE cannot).
        catP = [spool.tile([128, N], ATT, tag=f"cat{p}", name=f"cat{p}")
                for p in range(2)]
        # out2 staging at partition base 0 (pre-add input for the tails)
        out2sb = [spool.tile([64, N], F16, tag=f"o2{h}", name=f"o2{h}")
                  for h in range(HG)]

        wmt = spool.tile([128, 512], ATT, tag="wmt", name="wmt")

        # ============ Scope 1: all projections + channel-attn logits ======
        with tc.tile_pool(name="proj_in", bufs=1) as ipool, \
             tc.tile_pool(name="psp", bufs=4, space="PSUM") as psp, \
             tc.tile_pool(name="pscm", bufs=1, space="PSUM") as pscm:
            # ---- HAM warmup: matmuls on a memset tile keep the PE's
            # activity window busy from t~=0 while the input DMAs ramp up
            # (the first ~10us of dynamic-DMA traffic trickles).
            nc.vector.memset(wmt[:], 0.03125)
            for w in range(N_WARMUP):
                wps = psp.tile([128, 512], F32, tag="pj", name=f"warm{w}")
                nc.tensor.matmul(
                    wps[:], lhsT=wmt[:, 0:128], rhs=wmt[:],
                    start=True, stop=True,
                )

            # weights + inputs in consumption order: z first (z1T), then
            # x (xh), then z2/cm weights, then y (yhT).
            wsa1_t = [ipool.tile([128, CIN], BF16, tag=f"wsa1_{k}",
                                 name=f"wsa1_{k}") for k in range(KC)]
            wse1_t = [ipool.tile([128, CIN], BF16, tag=f"wse1_{k}",
                                 name=f"wse1_{k}") for k in range(KC)]
            wse2_t = [ipool.tile([128, CIN], BF16, tag=f"wse2_{k}",
                                 name=f"wse2_{k}") for k in range(KC)]
            wsa2_t = [ipool.tile([128, CIN], BF16, tag=f"wsa2_{k}",
                                 name=f"wsa2_{k}") for k in range(KC)]
            xTt = [ipool.tile([128, N], BF16, tag=f"x{k}", name=f"x{k}")
                   for k in range(KC)]
            zTt = [ipool.tile([128, N], BF16, tag=f"z{k}", name=f"z{k}")
                   for k in range(KC)]
            yTt = [ipool.tile([128, N], BF16, tag=f"y{k}", name=f"y{k}")
                   for k in range(KC)]
            for k in range(KC):
                nc.sync.dma_start(wsa1_t[k][:], wsa1_d[k * 128:(k + 1) * 128, :])
            for k in range(KC):
                nc.sync.dma_start(zTt[k][:], zT_d[k * 128:(k + 1) * 128, :])
            for k in range(KC):
                nc.sync.dma_start(wse1_t[k][:], wse1_d[k * 128:(k + 1) * 128, :])
            for k in range(KC):
                nc.sync.dma_start(xTt[k][:], xT_d[k * 128:(k + 1) * 128, :])
            for k in range(KC):
                nc.sync.dma_start(wse2_t[k][:], wse2_d[k * 128:(k + 1) * 128, :])
                nc.sync.dma_start(wsa2_t[k][:], wsa2_d[k * 128:(k + 1) * 128, :])
            for k in range(KC):
                nc.sync.dma_start(yTt[k][:], yT_d[k * 128:(k + 1) * 128, :])

            # head-PAIR channel-logit tiles: head 2q+qq lives at
            # partitions 64*qq; the two heads' matmuls col-tile and run
            # concurrently on the PE
            cmps = [pscm.tile([128, DH], F32, tag=f"cmp{q}", name=f"cmp{q}")
                    for q in range(2)]

            # --- z1T (transposed projection, pre-scaled by FE_ALPHA) ---
            for m in range(2):
                for nb in range(4):
                    ps = psp.tile([128, 512], F32, tag="pj", name=f"psz{m}{nb}")
                    for k in range(KC):
                        nc.tensor.matmul(
                            ps[:],
                            lhsT=wsa1_t[k][:, m * 128:(m + 1) * 128],
                            rhs=zTt[k][:, nb * 512:(nb + 1) * 512],
                            start=(k == 0), stop=(k == KC - 1),
                        )
                    nc.scalar.mul(z1T[m][:, nb * 512:(nb + 1) * 512], ps[:],
                                  FE_ALPHA)

            # --- xh (natural, augmented with ones) ---
            for i in range(NCH):
                ps = psp.tile([128, 512], F32, tag="pj", name=f"psx{i}")
                for k in range(KC):
                    nc.tensor.matmul(
                        ps[:, 0:CIN],
                        lhsT=xTt[k][:, i * 128:(i + 1) * 128],
                        rhs=wse1_t[k][:],
                        start=(k == 0), stop=(k == KC - 1),
                    )
                src = ps[:, 0:CIN].rearrange("p (h c) -> p h c", c=DH)
                dst = xh_aug[i][:].rearrange("p (h c) -> p h c", c=DH + 1)
                nc.vector.tensor_copy(dst[:, :, 0:DH], src)
                nc.scalar.activation(dst[:, :, DH:DH + 1], src[:, :, 0:1],
                                     mybir.ActivationFunctionType.Copy,
                                     bias=1.0, scale=0.0)

            # --- z2 (natural, streamed) + channel-attn logits ---
            for i in range(NCH):
                ps2 = psp.tile([128, 512], F32, tag="pj", name=f"psz2_{i}")
                for k in range(KC):
                    nc.tensor.matmul(
                        ps2[:, 0:CIN],
                        lhsT=zTt[k][:, i * 128:(i + 1) * 128],
                        rhs=wse2_t[k][:],
                        start=(k == 0), stop=(k == KC - 1),
                    )
                z2n = ipool.tile([128, CIN], ATT, tag="z2n", bufs=3,
                                 name=f"z2n{i}")
                nc.scalar.copy(z2n[:], ps2[:, 0:CIN])
                for h in range(HG):
                    off = 64 * (h % 2)
                    nc.tensor.matmul(
                        cmps[h // 2][off:off + DH, :],
                        lhsT=xh_aug[i][:, 65 * h:65 * h + DH],
                        rhs=z2n[:, DH * h:DH * (h + 1)],
                        start=(i == 0), stop=(i == NCH - 1),
                    )

            # --- yhT (transposed projection) ---
            for m in range(2):
                for nb in range(4):
                    ps = psp.tile([128, 512], F32, tag="pj", name=f"psy{m}{nb}")
                    for k in range(KC):
                        nc.tensor.matmul(
                            ps[:],
                            lhsT=wsa2_t[k][:, m * 128:(m + 1) * 128],
                            rhs=yTt[k][:, nb * 512:(nb + 1) * 512],
                            start=(k == 0), stop=(k == KC - 1),
                        )
                    nc.scalar.copy(yhT[m][:, nb * 512:(nb + 1) * 512], ps[:])

            # --- channel-attn softmax, DMA'd into pair-packed secm_sb ---
            # (engines are partition-locked and the custom recip only works
            # at base 0, so odd heads bounce their sums through partition 0
            # via tiny DMAs)
            for q in range(2):
                st = ipool.tile([128, DH], ATT, tag="cmstage", bufs=2,
                                name=f"cmstage{q}")
                for hh in range(2):
                    h, off = 2 * q + hh, 64 * hh
                    nc.scalar.activation(st[off:off + 64, :],
                                         cmps[q][off:off + 64, :], EXP,
                                         scale=CM_SCALE,
                                         accum_out=rs[q][off:off + 64, 0:1])
                nc.vector.reciprocal_approx_fast(rcm[q][0:64, 0:1],
                                                 rs[q][0:64, 0:1])
                nc.sync.dma_start(rs0[0:64, 0:1], rs[q][64:128, 0:1])
                nc.vector.reciprocal_approx_fast(rcm0[0:64, 0:1],
                                                 rs0[0:64, 0:1])
                nc.sync.dma_start(rcm[q][64:128, 0:1], rcm0[0:64, 0:1])
                for hh in range(2):
                    h, off = 2 * q + hh, 64 * hh
                    nc.vector.tensor_scalar_mul(st[off:off + 64, :],
                                                st[off:off + 64, :],
                                                rcm[q][off:off + 64, 0:1])
                    nc.sync.dma_start(secm_sb[h // 2][64 * (h % 2):
                                                      64 * (h % 2) + 64, :],
                                      st[off:off + 64, :])

        # ============ Scope 2: out2, spatial attention, final projection ==
        # PSUM: S tag 2x[128,1024] (4 banks) + av 2x[128,512] (2 banks) +
        # aux 2x[128,512] (2 banks) = 8 banks exactly.
        with tc.tile_pool(name="psS", bufs=2, space="PSUM") as psS, \
             tc.tile_pool(name="psAV", bufs=2, space="PSUM") as psAV, \
             tc.tile_pool(name="psaux", bufs=2, space="PSUM") as psaux:

            # Aux matmul stream: out2 + final-projection matmuls, one PE
            # instruction per thunk, drained inside the spatial j-loops so
            # the PE always has ready work while ScalarE/VectorE run exps.
            aux_thunks = []
            final_psf = {}

            def emit_out2(h, nb):
                p_, off = h // 2, 64 * (h % 2)
                pso = psaux.tile([128, 512], F32, tag="aux",
                                 name=f"pso{h}{nb}")
                mm = nc.tensor.matmul(
                    pso[0:64, :],
                    lhsT=secm_sb[p_][off:off + 64, :],
                    rhs=yhT[p_][off:off + 64, nb * 512:(nb + 1) * 512],
                    start=True, stop=True,
                )
                nc.scalar.copy(
                    out2sb[h][:, nb * 512:(nb + 1) * 512], pso[0:64, :])
                return mm

            def emit_final_mm(d, nb, p):
                if p == 0:
                    final_psf[(d, nb)] = psaux.tile(
                        [128, 512], F32, tag="aux", name=f"psf{d}{nb}")
                psf = final_psf[(d, nb)]
                mm = nc.tensor.matmul(
                    psf[:],
                    lhsT=wqP[p][:, d * 128:(d + 1) * 128],
                    rhs=catP[p][:, nb * 512:(nb + 1) * 512],
                    start=(p == 0), stop=(p == 1),
                )
                if p == 1:
                    ob = opool.tile([128, 512], F16, tag="ob",
                                    name=f"ob{d}{nb}")
                    nc.scalar.copy(ob[:], psf[:])
                    nc.sync.dma_start(
                        outT_d[d * 128:(d + 1) * 128,
                               nb * 512:(nb + 1) * 512],
                        ob[:],
                    )
                return mm

            for h in range(HG):
                for nb in range(4):
                    aux_thunks.append(lambda h=h, nb=nb: emit_out2(h, nb))

            def queue_finals(nb, ds=range(8)):
                for d in ds:
                    for p in range(2):
                        aux_thunks.append(
                            lambda d=d, nb=nb, p=p: emit_final_mm(d, nb, p))

            def drain_aux(k, anchor=None):
                # anchor pins the aux matmul into this drain slot's position
                # in the PE stream - the scheduler's gap-filler otherwise
                # hoists finals into earlier windows where their catP inputs
                # are still several microseconds from ready
                for _ in range(k):
                    if aux_thunks:
                        mm = aux_thunks.pop(0)()
                        if anchor is not None and mm is not None:
                            add_dep_helper(mm.ins, anchor.ins, sync=False,
                                           reason="pin aux to drain slot")

            def make_tail(p_, ib, avs, ptt_last):
                # AV for the last j-pair + normalization tails; emitted at
                # the START of the next iteration so that iteration's S
                # matmuls sit ahead of it in the PE stream.
                icol = ib * 512

                def emit(anchor=None, vpin1=None, vpin2=None, spin=None):
                    def vpin(inst, tgt):
                        if tgt is not None:
                            add_dep_helper(inst.ins, tgt.ins, sync=False,
                                           reason="tail op behind exp stream")
                    last_avs = []
                    for hh in range(2):
                        h = 2 * p_ + hh
                        mm = nc.tensor.matmul(
                            avs[hh][0:DH + 1, :],
                            lhsT=xh_aug[NCH - 1][:, 65 * h:65 * h + DH + 1],
                            rhs=ptt_last[:, 512 * hh:512 * hh + 512],
                            start=False, stop=True,
                        )
                        if anchor is not None:
                            add_dep_helper(mm.ins, anchor.ins, sync=False,
                                           reason="tail AV after S")
                        last_avs.append(mm)
                    avsbs, rcs, bcs = [], [], []
                    for hh in range(2):
                        avsb = tpool.tile([DH + 1, 512], F32, tag="avsb",
                                          name=f"avsb{p_}{ib}{hh}")
                        cp = nc.vector.tensor_copy(avsb[:],
                                                   avs[hh][0:DH + 1, :])
                        vpin(cp, vpin1)
                        avsbs.append(avsb)
                    dens = []
                    for hh in range(2):
                        # custom-DVE recip and partition_broadcast both
                        # require base partition 0 on HW: DMA-shift the
                        # denominator row down first (2 KB, cheap)
                        den = tpool.tile([1, 512], F32, tag="den", bufs=2,
                                         name=f"den{p_}{ib}{hh}")
                        nc.sync.dma_start(den[:], avsbs[hh][DH:DH + 1, :])
                        dens.append(den)
                    for hh in range(2):
                        rc = tpool.tile([1, 512], F32, tag="rc", bufs=2,
                                        name=f"rc{p_}{ib}{hh}")
                        rci = nc.vector.reciprocal_approx_fast(
                            rc[:], dens[hh][:])
                        vpin(rci, vpin1)
                        rcs.append(rc)
                    for hh in range(2):
                        bc = tpool.tile([64, 512], F32, tag="bc", bufs=2,
                                        name=f"bc{p_}{ib}{hh}")
                        nc.gpsimd.partition_broadcast(bc[:], rcs[hh][:])
                        bcs.append(bc)
                    for hh in range(2):
                        h = 2 * p_ + hh
                        off = 64 * hh
                        tmp = tpool.tile([64, 512], F32, tag="tmp", bufs=2,
                                         name=f"tmp{p_}{ib}{hh}")
                        # mul/add on DVE (gpsimd only ever runs
                        # partition_broadcast: mixing op families there
                        # forces ~6us ucode library swaps every iteration),
                        # pinned behind a later exp so the DVE FIFO never
                        # parks them in front of the exp stream
                        mu = nc.vector.tensor_mul(tmp[:], avsbs[hh][0:DH, :],
                                                  bcs[hh][:])
                        vpin(mu, vpin2)
                        o2 = out2sb[h][:, icol:icol + 512]
                        if hh == 0:
                            ad = nc.vector.tensor_add(
                                catP[p_][0:64, icol:icol + 512], tmp[:], o2)
                        else:
                            # sum at base 0, DMA shifts it to partitions
                            # 64:128 of the pair tile
                            sm = tpool.tile([64, 512], ATT, tag="sm", bufs=2,
                                            name=f"sm{p_}{ib}")
                            ad = nc.vector.tensor_add(sm[:], tmp[:], o2)
                            nc.sync.dma_start(
                                catP[p_][64:128, icol:icol + 512], sm[:])
                        vpin(ad, vpin2)
                    return last_avs
                return emit

            pending_tail = None
            # --- spatial attention: iterations (ib 512-block, pair),
            #     key chunks j; the two heads' S matmuls are row-tiled
            #     (base partitions 0/64) so they run concurrently ---
            for ib in range(4):
                for p_ in range(2):
                    # nb's catP block is complete once BOTH pairs' tails
                    # ran; the second pair's tails execute during
                    # (ib+1, p0), so finals(nb) join the queue at (ib+1,p1)
                    if p_ == 1 and ib >= 1:
                        queue_finals(ib - 1)
                    icol = ib * 512
                    avs = [psAV.tile([128, 512], F32, tag="av",
                                     name=f"av{p_}{ib}{q}") for q in range(2)]
                    ptts = [None] * NCH
                    vexps = []
                    sexps = []
                    for j in range(NCH):  # key chunks
                        spt = psS.tile([128, 1024], F32, tag="S",
                                       name=f"S{p_}{ib}{j}")
                        s_anchor = None
                        for hh in range(2):
                            off = 64 * hh
                            s_anchor = nc.tensor.matmul(
                                spt[:, 512 * hh:512 * hh + 512],
                                lhsT=yhT[p_][off:off + 64,
                                             j * 128:(j + 1) * 128],
                                rhs=z1T[p_][off:off + 64, icol:icol + 512],
                                start=True, stop=True,
                            )
                        # separate buffer rings per exp engine: a shared
                        # ring serializes slot acquisition between the
                        # scalar and vector exp streams
                        ptt = ptpool.tile([128, 1024], ATT,
                                          tag="ptv" if _exp_on_dve(j) else "pts",
                                          name=f"pt{p_}{ib}{j}")
                        if _exp_on_dve(j):
                            vexps.append(nc.vector.tensor_scalar(
                                ptt[:].bitcast(I16), spt[:],
                                FE_B, None, mybir.AluOpType.add))
                        else:
                            sexps.append(nc.scalar.activation(
                                ptt[:], spt[:], EXP, scale=LN2_128))
                        ptts[j] = ptt
                        if j == 3 and pending_tail is not None:
                            pending_tail(s_anchor, vexps[0], vexps[1],
                                         sexps[-1] if sexps else None)
                            pending_tail = None
                        drain_aux(1, s_anchor)
                        if j > 0:
                            for hh in range(2):
                                h = 2 * p_ + hh
                                av_mm = nc.tensor.matmul(
                                    avs[hh][0:DH + 1, :],
                                    lhsT=xh_aug[j - 1][:, 65 * h:65 * h + DH + 1],
                                    rhs=ptts[j - 1][:, 512 * hh:512 * hh + 512],
                                    start=(j == 1), stop=False,
                                )
                                # force the PE static order [S(j), .., AV(j-1)]:
                                # an exp's cumulative matmul-counter wait only
                                # covers MMs ordered BEFORE its S pair, so the
                                # AVs must come after S or every exp transitively
                                # serializes behind the previous one
                                add_dep_helper(av_mm.ins, s_anchor.ins,
                                               sync=False,
                                               reason="AV after S in PE order")
                    pending_tail = make_tail(p_, ib, avs, ptts[NCH - 1])
            last_avs = pending_tail()
            # keep-warm fillers: the last tail chain leaves the PE idle for
            # ~5us which would re-throttle HAM right before the final
            # projection matmuls; pin some dummy matmuls behind the last AV
            for w in range(N_TAILFILL):
                fps = psS.tile([128, 1024], F32, tag="S", name=f"tfill{w}")
                mm = nc.tensor.matmul(
                    fps[:, 0:512], lhsT=wmt[:, 0:128], rhs=wmt[:],
                    start=True, stop=True,
                )
                if last_avs:
                    add_dep_helper(mm.ins, last_avs[-1].ins, sync=False,
                                   reason="tail keep-warm")
            queue_finals(3)
            drain_aux(len(aux_thunks))

    nc.compile()
    return nc


_NC_CACHE = {}


def _get_program():
    if "nc" not in _NC_CACHE:
        _NC_CACHE["nc"] = _build_program()
    return _NC_CACHE["nc"]


def _prep_input_maps(x, y, z, w_sa1, w_sa2, w_se1, w_se2, w_out):
    bf16 = lambda a: np.ascontiguousarray(
        np.asarray(a, dtype=np.float32).astype(ml_dtypes.bfloat16))
    maps = []
    for c in range(NCORES):
        b, g = divmod(c, G)
        sl = slice(g * CIN, (g + 1) * CIN)
        maps.append({
            "xT": bf16(np.asarray(x)[b].T),
            "yT": bf16(np.asarray(y)[b].T),
            "zT": bf16(np.asarray(z)[b].T),
            "w_sa1": bf16(np.asarray(w_sa1)[:, sl]),
            "w_sa2": bf16(np.asarray(w_sa2)[:, sl]),
            "w_se1": bf16(np.asarray(w_se1)[:, sl]),
            "w_se2": bf16(np.asarray(w_se2)[:, sl]),
            "w_out": bf16(np.asarray(w_out)[sl, :]),
        })
    return maps


def run(inputs, trace=False, trace_kwargs=None):
    """Run on hardware; returns (full_output, BassKernelResults)."""
    nc = _get_program()
    in_maps = _prep_input_maps(
        inputs["x"], inputs["y"], inputs["z"],
        inputs["w_sa1"], inputs["w_sa2"], inputs["w_se1"], inputs["w_se2"],
        inputs["w_out"],
    )
    res = run_bass_kernel_spmd(
        nc, in_maps, list(range(NCORES)), trace=trace,
        trace_kwargs=trace_kwargs or {},
    )
    out = np.zeros((B, N, DIM), dtype=np.float32)
    for c in range(NCORES):
        b, _g = divmod(c, G)
        out[b] += np.asarray(res.results[c]["outT"], dtype=np.float32).T
    out += np.asarray(inputs["b_out"], dtype=np.float32)
    return out, res


def kernel(**inputs) -> np.ndarray:
    out, _ = run(inputs, trace=False)
    return out


# revision 39
# speedup vs baseline: 1.1491x; 1.1491x over previous
"""Trainium2 Bass kernel for nn_Attention_81037442941065.

Dual-attention module (spatial [b,h,n,n] + channel [b,h,d,d]) with
B=2, N=2048, DIM=1024, 16 heads of d=64.

Sharding: 8 cores = (2 batches) x (4 head-groups of 4 heads).
Each core computes its batch/head-group slice end-to-end and produces a
partial (over head groups) output projection; the host sums the 4 group
partials per batch (the "all-reduce after to_out") and adds b_out.

Key engine balance (per core the spatial softmax needs 16.8M exps; the
ScalarE ACT unit does 1 elem/cycle/lane => ~110us if done there alone,
which was the old bottleneck):
  - exp is SPLIT between ScalarE (exact, activation Exp) and VectorE
    (fast-exp bit trick: P = bitcast_bf16(int16(round(S' + B))) where
    S' = S * 128*log2(e)*scale is pre-scaled by folding the constant
    into the z1 projection output. +-3% multiplicative ripple, zero mean
    in log space; softmax normalization cancels the common mode.
  - 1/den uses the custom-DVE reciprocal_approx_fast (the exact
    nc.vector.reciprocal took 3.3us per [1,512] call).
  - final projection contracts head-PAIRS: cat/w_out stored as
    [128, .] pair-tiles so the contraction K=128 (full PE rows),
    halving the number of projection matmuls.
  - ~44 warmup matmuls on the (already loaded) w_out tiles keep the PE
    HAM activity monitor busy during the initial input DMA so the whole
    kernel runs at 2.4 GHz instead of spending its first ~37us at 1.2.
  - outputs are fp16 partials (halves the output DMA; host sums in f32).

Per-core layouts (everything "T" is [channels, tokens]):
  z1T, yhT   : [256, 2048] as 2 tiles [128, 2048] (head pair per tile);
               z1T is pre-scaled by FE_ALPHA (see above)
  xh_aug     : 16 tiles [128, 260] (natural layout per 128-token chunk;
               per head 65 cols = 64 channels + a ones column so the AV
               matmul also produces the softmax denominators)
  spatial    : S^T = yh @ z1'^T computed [keys, queries] as row-tiled
               concurrent head-pair matmuls; exp split ScalarE/VectorE;
               AV matmul lhsT=[xh|1] accumulates over key chunks ->
               rows 0..63 = unnormalized out1^T, row 64 = sum of exp.
  channel    : [64,64] per head, softmax via Exp+accum_out and
               reciprocal_approx_fast.
"""

import sys

for _p in ("/opt/trn_rl_repo", "/opt/pypackages"):
    if _p not in sys.path:
        sys.path.insert(0, _p)

import ml_dtypes
import numpy as np
from contextlib import ExitStack

import concourse.bacc as bacc
import concourse.mybir as mybir
import concourse.tile as tile
from concourse.tile import add_dep_helper
from concourse.bass_utils import run_bass_kernel_spmd

F32 = mybir.dt.float32
F16 = mybir.dt.float16
I16 = mybir.dt.int16
BF16 = mybir.dt.bfloat16
ATT = mybir.dt.bfloat16   # attention-internal matmul dtype
EXP = mybir.ActivationFunctionType.Exp

B, N, DIM = 2, 2048, 1024
HEADS, DH = 16, 64
G = 4              # head groups == cores per batch
HG = HEADS // G    # heads per group (4)
CIN = HG * DH      # inner channels per core (256)
NCORES = 8
KC = DIM // 128    # contraction chunks for projections (8)
NCH = N // 128     # 128-token chunks (16)
SCALE = DH ** -0.5            # 1/8
CM_SCALE = SCALE / (N / DH)   # 1/256

# fast-exp constants: S' = FE_ALPHA * S accumulates in PSUM; then
#   ScalarE path: exp(LN2_128 * S') == exp(SCALE * S)
#   VectorE path: bitcast_bf16(int16(S' + FE_B)) ~= exp(SCALE * S)
FE_ALPHA = 128.0 * 1.4426950408889634 * SCALE
LN2_128 = 0.6931471805599453 / 128.0
FE_B = 16256.0 - 7.33        # mean-zero-in-log mantissa tweak

# exp tile assignment: whole [128,1024] tiles (per-instruction overhead
# is ~270ns, so wider is cheaper) alternate by j parity: even j ->
# ScalarE exact exp, odd j -> VectorE fast-exp.
def _exp_on_dve(j):
    return j % 2 == 1 and j != 15

N_WARMUP = 60                # HAM warmup matmuls at kernel start
N_TAILFILL = 34              # keep-warm matmuls during the last tail chain


def _build_program():
    nc = bacc.Bacc(
        "TRN2", target_bir_lowering=False, debug=False, num_devices=NCORES
    )

    # ---- DRAM I/O ----
    xT_d = nc.dram_tensor("xT", [DIM, N], BF16, kind="ExternalInput").ap()
    yT_d = nc.dram_tensor("yT", [DIM, N], BF16, kind="ExternalInput").ap()
    zT_d = nc.dram_tensor("zT", [DIM, N], BF16, kind="ExternalInput").ap()
    wsa1_d = nc.dram_tensor("w_sa1", [DIM, CIN], BF16, kind="ExternalInput").ap()
    wsa2_d = nc.dram_tensor("w_sa2", [DIM, CIN], BF16, kind="ExternalInput").ap()
    wse1_d = nc.dram_tensor("w_se1", [DIM, CIN], BF16, kind="ExternalInput").ap()
    wse2_d = nc.dram_tensor("w_se2", [DIM, CIN], BF16, kind="ExternalInput").ap()
    wout_d = nc.dram_tensor("w_out", [CIN, DIM], ATT, kind="ExternalInput").ap()
    outT_d = nc.dram_tensor("outT", [DIM, N], F16, kind="ExternalOutput").ap()

    with tile.TileContext(nc) as tc, ExitStack() as ctx:
        ppool = ctx.enter_context(tc.tile_pool(name="persist", bufs=1))

        # Persistent projection outputs (live across both scopes).
        z1T = [ppool.tile([128, N], ATT, tag=f"z1T{m}", name=f"z1T{m}")
               for m in range(2)]
        yhT = [ppool.tile([128, N], ATT, tag=f"yhT{m}", name=f"yhT{m}")
               for m in range(2)]
        xh_aug = [ppool.tile([128, HG * (DH + 1)], ATT, tag=f"xa{i}",
                             name=f"xa{i}") for i in range(NCH)]
        secm_sb = [ppool.tile([128, DH], ATT, tag=f"cm{p}", name=f"cm{p}")
                   for p in range(2)]
        rs = [ppool.tile([64, 1], F32, tag=f"rs{h}", name=f"rs{h}")
              for h in range(HG)]
        rcm = [ppool.tile([64, 1], F32, tag=f"rcm{h}", name=f"rcm{h}")
               for h in range(HG)]

        ptpool = ctx.enter_context(tc.tile_pool(name="pt", bufs=3))
        tpool = ctx.enter_context(tc.tile_pool(name="tails", bufs=3))
        opool = ctx.enter_context(tc.tile_pool(name="oout", bufs=3))
        spool = ctx.enter_context(tc.tile_pool(name="spat", bufs=1))
        # w_out as two head-pair tiles [128, DIM] matching catP below;
        # DMA'd first so they double as the HAM-warmup matmul operands.
        wqP = [spool.tile([128, DIM], ATT, tag=f"wq{p}", name=f"wq{p}")
               for p in range(2)]
        for p in range(2):
            nc.sync.dma_start(wqP[p][:], wout_d[p * 128:(p + 1) * 128, :])
        # cat^T staging as head-PAIR tiles [128, N]: head 2p+hh occupies
        # partitions 64*hh..64*hh+64.  The final projection contracts a
        # pair in ONE K=128 matmul.  DVE lanes are partition-locked, so
        # the odd head's blocks are computed at base 0 and DMA'd into
        # partitions 64:128 (DMA can shift partitions; DVE cannot).
        catP = [spool.tile([128, N], ATT, tag=f"cat{p}", name=f"cat{p}")
                for p in range(2)]
        # out2 staging at partition base 0 (pre-add input for the tails)
        out2sb = [spool.tile([64, N], F16, tag=f"o2{h}", name=f"o2{h}")
                  for h in range(HG)]

        wmt = spool.tile([128, 512], ATT, tag="wmt", name="wmt")

        # ============ Scope 1: all projections + channel-attn logits ======
        with tc.tile_pool(name="proj_in", bufs=1) as ipool, \
             tc.tile_pool(name="psp", bufs=4, space="PSUM") as psp, \
             tc.tile_pool(name="pscm", bufs=1, space="PSUM") as pscm:
            # ---- HAM warmup: matmuls on a memset tile keep the PE's
            # activity window busy from t~=0 while the input DMAs ramp up
            # (the first ~10us of dynamic-DMA traffic trickles).
            nc.vector.memset(wmt[:], 0.03125)
            for w in range(N_WARMUP):
                wps = psp.tile([128, 512], F32, tag="pj", name=f"warm{w}")
                nc.tensor.matmul(
                    wps[:], lhsT=wmt[:, 0:128], rhs=wmt[:],
                    start=True, stop=True,
                )

            # weights + inputs in consumption order: z first (z1T), then
            # x (xh), then z2/cm weights, then y (yhT).
            wsa1_t = [ipool.tile([128, CIN], BF16, tag=f"wsa1_{k}",
                                 name=f"wsa1_{k}") for k in range(KC)]
            wse1_t = [ipool.tile([128, CIN], BF16, tag=f"wse1_{k}",
                                 name=f"wse1_{k}") for k in range(KC)]
            wse2_t = [ipool.tile([128, CIN], BF16, tag=f"wse2_{k}",
                                 name=f"wse2_{k}") for k in range(KC)]
            wsa2_t = [ipool.tile([128, CIN], BF16, tag=f"wsa2_{k}",
                                 name=f"wsa2_{k}") for k in range(KC)]
            xTt = [ipool.tile([128, N], BF16, tag=f"x{k}", name=f"x{k}")
                   for k in range(KC)]
            zTt = [ipool.tile([128, N], BF16, tag=f"z{k}", name=f"z{k}")
                   for k in range(KC)]
            yTt = [ipool.tile([128, N], BF16, tag=f"y{k}", name=f"y{k}")
                   for k in range(KC)]
            for k in range(KC):
                nc.sync.dma_start(wsa1_t[k][:], wsa1_d[k * 128:(k + 1) * 128, :])
            for k in range(KC):
                nc.sync.dma_start(zTt[k][:], zT_d[k * 128:(k + 1) * 128, :])
            for k in range(KC):
                nc.sync.dma_start(wse1_t[k][:], wse1_d[k * 128:(k + 1) * 128, :])
            for k in range(KC):
                nc.sync.dma_start(xTt[k][:], xT_d[k * 128:(k + 1) * 128, :])
            for k in range(KC):
                nc.sync.dma_start(wse2_t[k][:], wse2_d[k * 128:(k + 1) * 128, :])
                nc.sync.dma_start(wsa2_t[k][:], wsa2_d[k * 128:(k + 1) * 128, :])
            for k in range(KC):
                nc.sync.dma_start(yTt[k][:], yT_d[k * 128:(k + 1) * 128, :])

            cmps = [pscm.tile([64, DH], F32, tag=f"cmp{h}", name=f"cmp{h}")
                    for h in range(HG)]

            # --- z1T (transposed projection, pre-scaled by FE_ALPHA) ---
            for m in range(2):
                for nb in range(4):
                    ps = psp.tile([128, 512], F32, tag="pj", name=f"psz{m}{nb}")
                    for k in range(KC):
                        nc.tensor.matmul(
                            ps[:],
                            lhsT=wsa1_t[k][:, m * 128:(m + 1) * 128],
                            rhs=zTt[k][:, nb * 512:(nb + 1) * 512],
                            start=(k == 0), stop=(k == KC - 1),
                        )
                    nc.scalar.mul(z1T[m][:, nb * 512:(nb + 1) * 512], ps[:],
                                  FE_ALPHA)

            # --- xh (natural, augmented with ones) ---
            for i in range(NCH):
                ps = psp.tile([128, 512], F32, tag="pj", name=f"psx{i}")
                for k in range(KC):
                    nc.tensor.matmul(
                        ps[:, 0:CIN],
                        lhsT=xTt[k][:, i * 128:(i + 1) * 128],
                        rhs=wse1_t[k][:],
                        start=(k == 0), stop=(k == KC - 1),
                    )
                src = ps[:, 0:CIN].rearrange("p (h c) -> p h c", c=DH)
                dst = xh_aug[i][:].rearrange("p (h c) -> p h c", c=DH + 1)
                nc.vector.tensor_copy(dst[:, :, 0:DH], src)
                nc.scalar.activation(dst[:, :, DH:DH + 1], src[:, :, 0:1],
                                     mybir.ActivationFunctionType.Copy,
                                     bias=1.0, scale=0.0)

            # --- z2 (natural, streamed) + channel-attn logits ---
            for i in range(NCH):
                ps2 = psp.tile([128, 512], F32, tag="pj", name=f"psz2_{i}")
                for k in range(KC):
                    nc.tensor.matmul(
                        ps2[:, 0:CIN],
                        lhsT=zTt[k][:, i * 128:(i + 1) * 128],
                        rhs=wse2_t[k][:],
                        start=(k == 0), stop=(k == KC - 1),
                    )
                z2n = ipool.tile([128, CIN], ATT, tag="z2n", bufs=3,
                                 name=f"z2n{i}")
                nc.scalar.copy(z2n[:], ps2[:, 0:CIN])
                for h in range(HG):
                    nc.tensor.matmul(
                        cmps[h][:],
                        lhsT=xh_aug[i][:, 65 * h:65 * h + DH],
                        rhs=z2n[:, DH * h:DH * (h + 1)],
                        start=(i == 0), stop=(i == NCH - 1),
                    )

            # --- yhT (transposed projection) ---
            for m in range(2):
                for nb in range(4):
                    ps = psp.tile([128, 512], F32, tag="pj", name=f"psy{m}{nb}")
                    for k in range(KC):
                        nc.tensor.matmul(
                            ps[:],
                            lhsT=wsa2_t[k][:, m * 128:(m + 1) * 128],
                            rhs=yTt[k][:, nb * 512:(nb + 1) * 512],
                            start=(k == 0), stop=(k == KC - 1),
                        )
                    nc.scalar.copy(yhT[m][:, nb * 512:(nb + 1) * 512], ps[:])

            # --- channel-attn softmax, DMA'd into pair-packed secm_sb ---
            for h in range(HG):
                p_, off = h // 2, 64 * (h % 2)
                st = ipool.tile([64, DH], ATT, tag="cmstage", bufs=4,
                                name=f"cmstage{h}")
                nc.scalar.activation(st[:], cmps[h][:], EXP,
                                     scale=CM_SCALE,
                                     accum_out=rs[h][0:64, 0:1])
                nc.vector.reciprocal_approx_fast(rcm[h][0:64, 0:1],
                                                 rs[h][0:64, 0:1])
                nc.vector.tensor_scalar_mul(st[:], st[:], rcm[h][0:64, 0:1])
                nc.sync.dma_start(secm_sb[p_][off:off + 64, :], st[:])

        # ============ Scope 2: out2, spatial attention, final projection ==
        # PSUM: S tag 2x[128,1024] (4 banks) + av 2x[128,512] (2 banks) +
        # aux 2x[128,512] (2 banks) = 8 banks exactly.
        with tc.tile_pool(name="psS", bufs=2, space="PSUM") as psS, \
             tc.tile_pool(name="psAV", bufs=2, space="PSUM") as psAV, \
             tc.tile_pool(name="psaux", bufs=2, space="PSUM") as psaux:

            # Aux matmul stream: out2 + final-projection matmuls, one PE
            # instruction per thunk, drained inside the spatial j-loops so
            # the PE always has ready work while ScalarE/VectorE run exps.
            aux_thunks = []
            final_psf = {}

            def emit_out2(h, nb):
                p_, off = h // 2, 64 * (h % 2)
                pso = psaux.tile([128, 512], F32, tag="aux",
                                 name=f"pso{h}{nb}")
                mm = nc.tensor.matmul(
                    pso[0:64, :],
                    lhsT=secm_sb[p_][off:off + 64, :],
                    rhs=yhT[p_][off:off + 64, nb * 512:(nb + 1) * 512],
                    start=True, stop=True,
                )
                nc.scalar.copy(
                    out2sb[h][:, nb * 512:(nb + 1) * 512], pso[0:64, :])
                return mm

            def emit_final_mm(d, nb, p):
                if p == 0:
                    final_psf[(d, nb)] = psaux.tile(
                        [128, 512], F32, tag="aux", name=f"psf{d}{nb}")
                psf = final_psf[(d, nb)]
                mm = nc.tensor.matmul(
                    psf[:],
                    lhsT=wqP[p][:, d * 128:(d + 1) * 128],
                    rhs=catP[p][:, nb * 512:(nb + 1) * 512],
                    start=(p == 0), stop=(p == 1),
                )
                if p == 1:
                    ob = opool.tile([128, 512], F16, tag="ob",
                                    name=f"ob{d}{nb}")
                    nc.scalar.copy(ob[:], psf[:])
                    nc.sync.dma_start(
                        outT_d[d * 128:(d + 1) * 128,
                               nb * 512:(nb + 1) * 512],
                        ob[:],
                    )
                return mm

            for h in range(HG):
                for nb in range(4):
                    aux_thunks.append(lambda h=h, nb=nb: emit_out2(h, nb))

            def queue_finals(nb, ds=range(8)):
                for d in ds:
                    for p in range(2):
                        aux_thunks.append(
                            lambda d=d, nb=nb, p=p: emit_final_mm(d, nb, p))

            def drain_aux(k, anchor=None):
                # anchor pins the aux matmul into this drain slot's position
                # in the PE stream - the scheduler's gap-filler otherwise
                # hoists finals into earlier windows where their catP inputs
                # are still several microseconds from ready
                for _ in range(k):
                    if aux_thunks:
                        mm = aux_thunks.pop(0)()
                        if anchor is not None and mm is not None:
                            add_dep_helper(mm.ins, anchor.ins, sync=False,
                                           reason="pin aux to drain slot")

            def make_tail(p_, ib, avs, ptt_last):
                # AV for the last j-pair + normalization tails; emitted at
                # the START of the next iteration so that iteration's S
                # matmuls sit ahead of it in the PE stream.
                icol = ib * 512

                def emit(anchor=None, vpin1=None, vpin2=None, spin=None):
                    def vpin(inst, tgt):
                        if tgt is not None:
                            add_dep_helper(inst.ins, tgt.ins, sync=False,
                                           reason="tail op behind exp stream")
                    last_avs = []
                    for hh in range(2):
                        h = 2 * p_ + hh
                        mm = nc.tensor.matmul(
                            avs[hh][0:DH + 1, :],
                            lhsT=xh_aug[NCH - 1][:, 65 * h:65 * h + DH + 1],
                            rhs=ptt_last[:, 512 * hh:512 * hh + 512],
                            start=False, stop=True,
                        )
                        if anchor is not None:
                            add_dep_helper(mm.ins, anchor.ins, sync=False,
                                           reason="tail AV after S")
                        last_avs.append(mm)
                    avsbs, rcs, bcs = [], [], []
                    for hh in range(2):
                        avsb = tpool.tile([DH + 1, 512], F32, tag="avsb",
                                          name=f"avsb{p_}{ib}{hh}")
                        cp = nc.vector.tensor_copy(avsb[:],
                                                   avs[hh][0:DH + 1, :])
                        vpin(cp, vpin1)
                        avsbs.append(avsb)
                    dens = []
                    for hh in range(2):
                        # custom-DVE recip and partition_broadcast both
                        # require base partition 0 on HW: DMA-shift the
                        # denominator row down first (2 KB, cheap)
                        den = tpool.tile([1, 512], F32, tag="den", bufs=2,
                                         name=f"den{p_}{ib}{hh}")
                        nc.sync.dma_start(den[:], avsbs[hh][DH:DH + 1, :])
                        dens.append(den)
                    for hh in range(2):
                        rc = tpool.tile([1, 512], F32, tag="rc", bufs=2,
                                        name=f"rc{p_}{ib}{hh}")
                        rci = nc.vector.reciprocal_approx_fast(
                            rc[:], dens[hh][:])
                        vpin(rci, vpin1)
                        rcs.append(rc)
                    for hh in range(2):
                        bc = tpool.tile([64, 512], F32, tag="bc", bufs=2,
                                        name=f"bc{p_}{ib}{hh}")
                        nc.gpsimd.partition_broadcast(bc[:], rcs[hh][:])
                        bcs.append(bc)
                    for hh in range(2):
                        h = 2 * p_ + hh
                        off = 64 * hh
                        tmp = tpool.tile([64, 512], F32, tag="tmp", bufs=2,
                                         name=f"tmp{p_}{ib}{hh}")
                        # mul/add on DVE (gpsimd only ever runs
                        # partition_broadcast: mixing op families there
                        # forces ~6us ucode library swaps every iteration),
                        # pinned behind a later exp so the DVE FIFO never
                        # parks them in front of the exp stream
                        mu = nc.vector.tensor_mul(tmp[:], avsbs[hh][0:DH, :],
                                                  bcs[hh][:])
                        vpin(mu, vpin2)
                        o2 = out2sb[h][:, icol:icol + 512]
                        if hh == 0:
                            ad = nc.vector.tensor_add(
                                catP[p_][0:64, icol:icol + 512], tmp[:], o2)
                        else:
                            # sum at base 0, DMA shifts it to partitions
                            # 64:128 of the pair tile
                            sm = tpool.tile([64, 512], ATT, tag="sm", bufs=2,
                                            name=f"sm{p_}{ib}")
                            ad = nc.vector.tensor_add(sm[:], tmp[:], o2)
                            nc.sync.dma_start(
                                catP[p_][64:128, icol:icol + 512], sm[:])
                        vpin(ad, vpin2)
                    return last_avs
                return emit

            pending_tail = None
            # --- spatial attention: iterations (ib 512-block, pair),
            #     key chunks j; the two heads' S matmuls are row-tiled
            #     (base partitions 0/64) so they run concurrently ---
            for ib in range(4):
                for p_ in range(2):
                    # nb's catP block is complete once BOTH pairs' tails
                    # ran; the second pair's tails execute during
                    # (ib+1, p0), so finals(nb) join the queue at (ib+1,p1)
                    if p_ == 1 and ib >= 1:
                        queue_finals(ib - 1)
                    icol = ib * 512
                    avs = [psAV.tile([128, 512], F32, tag="av",
                                     name=f"av{p_}{ib}{q}") for q in range(2)]
                    ptts = [None] * NCH
                    vexps = []
                    sexps = []
                    for j in range(NCH):  # key chunks
                        spt = psS.tile([128, 1024], F32, tag="S",
                                       name=f"S{p_}{ib}{j}")
                        s_anchor = None
                        for hh in range(2):
                            off = 64 * hh
                            s_anchor = nc.tensor.matmul(
                                spt[:, 512 * hh:512 * hh + 512],
                                lhsT=yhT[p_][off:off + 64,
                                             j * 128:(j + 1) * 128],
                                rhs=z1T[p_][off:off + 64, icol:icol + 512],
                                start=True, stop=True,
                            )
                        # separate buffer rings per exp engine: a shared
                        # ring serializes slot acquisition between the
                        # scalar and vector exp streams
                        ptt = ptpool.tile([128, 1024], ATT,
                                          tag="ptv" if _exp_on_dve(j) else "pts",
                                          name=f"pt{p_}{ib}{j}")
                        if _exp_on_dve(j):
                            vexps.append(nc.vector.tensor_scalar(
                                ptt[:].bitcast(I16), spt[:],
                                FE_B, None, mybir.AluOpType.add))
                        else:
                            sexps.append(nc.scalar.activation(
                                ptt[:], spt[:], EXP, scale=LN2_128))
                        ptts[j] = ptt
                        if j == 3 and pending_tail is not None:
                            pending_tail(s_anchor, vexps[0], vexps[1],
                                         sexps[-1] if sexps else None)
                            pending_tail = None
                        drain_aux(1, s_anchor)
                        if j > 0:
                            for hh in range(2):
                                h = 2 * p_ + hh
                                av_mm = nc.tensor.matmul(
                                    avs[hh][0:DH + 1, :],
                                    lhsT=xh_aug[j - 1][:, 65 * h:65 * h + DH + 1],
                                    rhs=ptts[j - 1][:, 512 * hh:512 * hh + 512],
                                    start=(j == 1), stop=False,
                                )
                                # force the PE static order [S(j), .., AV(j-1)]:
                                # an exp's cumulative matmul-counter wait only
                                # covers MMs ordered BEFORE its S pair, so the
                                # AVs must come after S or every exp transitively
                                # serializes behind the previous one
                                add_dep_helper(av_mm.ins, s_anchor.ins,
                                               sync=False,
                                               reason="AV after S in PE order")
                    pending_tail = make_tail(p_, ib, avs, ptts[NCH - 1])
            last_avs = pending_tail()
            # keep-warm fillers: the last tail chain leaves the PE idle for
            # ~5us which would re-throttle HAM right before the final
            # projection matmuls; pin some dummy matmuls behind the last AV
            for w in range(N_TAILFILL):
                fps = psS.tile([128, 1024], F32, tag="S", name=f"tfill{w}")
                mm = nc.tensor.matmul(
                    fps[:, 0:512], lhsT=wmt[:, 0:128], rhs=wmt[:],
                    start=True, stop=True,
                )
                if last_avs:
                    add_dep_helper(mm.ins, last_avs[-1].ins, sync=False,
                                   reason="tail keep-warm")
            queue_finals(3)
            drain_aux(len(aux_thunks))

    nc.compile()
    return nc


_NC_CACHE = {}


def _get_program():
    if "nc" not in _NC_CACHE:
        _NC_CACHE["nc"] = _build_program()
    return _NC_CACHE["nc"]


def _prep_input_maps(x, y, z, w_sa1, w_sa2, w_se1, w_se2, w_out):
    bf16 = lambda a: np.ascontiguousarray(
        np.asarray(a, dtype=np.float32).astype(ml_dtypes.bfloat16))
    maps = []
    for c in range(NCORES):
        b, g = divmod(c, G)
        sl = slice(g * CIN, (g + 1) * CIN)
        maps.append({
            "xT": bf16(np.asarray(x)[b].T),
            "yT": bf16(np.asarray(y)[b].T),
            "zT": bf16(np.asarray(z)[b].T),
            "w_sa1": bf16(np.asarray(w_sa1)[:, sl]),
            "w_sa2": bf16(np.asarray(w_sa2)[:, sl]),
            "w_se1": bf16(np.asarray(w_se1)[:, sl]),
            "w_se2": bf16(np.asarray(w_se2)[:, sl]),
            "w_out": bf16(np.asarray(w_out)[sl, :]),
        })
    return maps


def run(inputs, trace=False, trace_kwargs=None):
    """Run on hardware; returns (full_output, BassKernelResults)."""
    nc = _get_program()
    in_maps = _prep_input_maps(
        inputs["x"], inputs["y"], inputs["z"],
        inputs["w_sa1"], inputs["w_sa2"], inputs["w_se1"], inputs["w_se2"],
        inputs["w_out"],
    )
    res = run_bass_kernel_spmd(
        nc, in_maps, list(range(NCORES)), trace=trace,
        trace_kwargs=trace_kwargs or {},
    )
    out = np.zeros((B, N, DIM), dtype=np.float32)
    for c in range(NCORES):
        b, _g = divmod(c, G)
        out[b] += np.asarray(res.results[c]["outT"], dtype=np.float32).T
    out += np.asarray(inputs["b_out"], dtype=np.float32)
    return out, res


def kernel(**inputs) -> np.ndarray:
    out, _ = run(inputs, trace=False)
    return out
